# revision 1
# baseline (speedup 1.0000x reference)
"""GAT kernel for Trainium2 (Bass/Tile), data-parallel over batch on 8 cores.

Per-core math (one batch element, N nodes, H heads, D=E=128). The softmax
numerator exp(lrelu(z_ij)) with z_ij = a_s_i + a_n_j is rewritten using
monotonicity of exp and the outer-sum structure of z:

  exp(lrelu(z)) = max(exp(z), exp(z/5))             (lrelu = max(z, 0.2z))
                = s_i * max(w_i * v_j, t_j) * const (divide by s_i=exp(.2 a_s_i),
                                                     cancels in softmax)
  with w = exp(0.8 a_s) (per-i), r = exp(-0.8 a_n), v = exp(a_n - 2) (per-j):
  p_ji = max(w_i, r_j) * adjT_ji     (v folded into the feature matrix)

so ALL transcendentals act on O(N) vectors instead of O(N^2) matrices.
Per 128xN chunk the score work is one DVE tensor_scalar (max, 4x mode) plus
one DVE tensor_tensor mask multiply (2x mode); a slice of chunks runs as a
fused scalar_tensor_tensor on the otherwise-idle GPSIMD engine. The w_i
broadcast row is replicated across partitions by a DMA round-trip through a
DRAM scratch row (partition_broadcast). Attention output accumulates in fp16
matmuls with fp32 PSUM; the rowsum falls out of the v column appended to the
v-scaled features, so softmax normalization is a per-partition reciprocal +
Relu(scale=rec) on the Scalar engine. Out is written fp16 and upcast on host.
"""

import os
import sys

sys.path.insert(0, "/opt/trn_rl_repo")

import numpy as np

import concourse.bass as bass
import concourse.bacc as bacc
import concourse.mybir as mybir
import concourse.tile as tile
from concourse.bass_utils import run_bass_kernel_spmd

F32 = mybir.dt.float32
F16 = mybir.dt.float16
P = 128


def build_core_program(N, H, D=128, E=128):
    """Trace the Bass program computing one batch element of the GAT."""
    nc = bacc.Bacc("TRN2", debug=False, target_bir_lowering=False)
    NCH = N // P  # node chunks
    EA = E + 1    # feat columns + v column (rowsum)
    SEG = 512     # max matmul moving-dim columns (one PSUM bank of fp32)
    segs = [(s, min(SEG, N - s)) for s in range(0, N, SEG)]
    HP = H // 2


    # wx = [kas | kaug | xT] packed on host (weights first)
    WXW = N + H * (E + 1) + H
    XOFF = H * (E + 1) + H
    wx = nc.dram_tensor("wx", [D, WXW], F16, kind="ExternalInput").ap()
    adjT = nc.dram_tensor("adjT", [N, N], F16, kind="ExternalInput").ap()
    out = nc.dram_tensor("out", [N, H * E], F16, kind="ExternalOutput").ap()

    with tile.TileContext(nc) as tc:
        with (
            tc.tile_pool(name="const", bufs=1) as const_pool,
            tc.tile_pool(name="xt", bufs=1) as xt_pool,
            tc.tile_pool(name="adj", bufs=1) as adj_pool,
            tc.tile_pool(name="fr", bufs=1) as fr_pool,
            tc.tile_pool(name="rv", bufs=1) as rv_pool,
            tc.tile_pool(name="wr", bufs=1) as wr_pool,
            tc.tile_pool(name="w16", bufs=1) as w16_pool,
        ):
            shift_sb = const_pool.tile([P, 1], F32, tag="shift")
            nc.vector.memset(shift_sb[:], -2.0)
            ones_sb = const_pool.tile([1, P], F16, tag="ones")
            nc.vector.memset(ones_sb[:], 1.0)
            warm_sb = const_pool.tile([P, 1], F32, tag="warm")
            nc.scalar.activation(warm_sb[:], shift_sb[:],
                                 mybir.ActivationFunctionType.Exp)

            # each dma_start lands on ONE ~22.5GB/s queue; slice every large
            # transfer into many dma_starts so they spread across all 16
            # queues and the critical-path inputs arrive in ~1-2us
            KSEG = min(int(os.environ.get("GAT_KSEG", "260")), XOFF)
            XSEG = min(int(os.environ.get("GAT_XSEG", "256")), N)
            ASEG = min(int(os.environ.get("GAT_ASEG", "512")), N)
            # separate tiles (not one packed tile) so dependency tracking is
            # fine-grained: the pa matmul must not wait for ALL wx DMAs
            kas_t = xt_pool.tile([D, H], F16, tag="kas")
            kaug_t = xt_pool.tile([D, H * EA], F16, tag="kaug")
            NXT = max(N // 512, 1)
            xt_t = [xt_pool.tile([D, min(512, N)], F16, tag=f"xt{i}",
                                 name=f"xt{i}") for i in range(NXT)]
            with tc.high_priority():
                nc.sync.dma_start(out=kas_t[:], in_=wx[:, 0:H])
                for s in range(0, N, XSEG):
                    ti, off = s // 512, s % 512
                    nc.sync.dma_start(
                        out=xt_t[ti][:, off:off + XSEG],
                        in_=wx[:, XOFF + s:XOFF + s + XSEG])
                for s in range(0, H * EA, KSEG):
                    e = min(H * EA, s + KSEG)
                    nc.sync.dma_start(out=kaug_t[:, s:e],
                                      in_=wx[:, H + s:H + e])
            kas_sb = kas_t[:]
            kaug_sb = kaug_t[:]

            def xt_cols(s0, w):
                ti, off = s0 // 512, s0 % 512
                assert off + w <= 512
                return xt_t[ti][:, off:off + w]

            # adjacency packed as quad tiles: adj4[c4][p, k*N + i] holds
            # adjT row (4*c4+k)*P + p, so one tensor_tensor masks 4 chunks.
            # DMAs for it are emitted AFTER the w16 broadcast chain below so
            # the broadcasts are not stuck behind 2MB of adjacency in the
            # DMA queues (adjT isn't consumed until well after w16).
            QC = 4 if NCH % 4 == 0 else 1
            NQ = NCH // QC
            adj_sb = [adj_pool.tile([P, QC * N], F16, tag=f"adj{c4}",
                                    name=f"adj{c4}")
                      for c4 in range(NQ)]

            # w16row[h, i] = exp(0.8 * a_s[h, i]) fp16, then broadcast each
            # row across all 128 partitions (PE outer product + ACT copy)
            w16row = wr_pool.tile([H, N], F16, tag="w16row")
            w16row0 = [wr_pool.tile([1, N], F16, tag=f"w16row0_{h}",
                                    name=f"w16row0_{h}") for h in range(1, H)]
            # w16[h][j, i] = w16row[h, i] for all j
            w16 = [w16_pool.tile([P, N], F16, tag=f"w16_{h}", name=f"w16_{h}")
                   for h in range(H)]

            # feat2v[hp][c]: [P, 2*EA] fp16 = [feat_h0 | 1 | feat_h1 | 1]
            # (v is folded into the tensor_scalar producing p, so the rowsum
            # column is plain ones)
            feat2v = [[fr_pool.tile([P, 2 * EA], F16, tag=f"fr{hp}_{c}",
                                    name=f"fr{hp}_{c}")
                       for c in range(NCH)] for hp in range(HP)]
            for hp in range(HP):
                for c in range(NCH):
                    ones_ap = feat2v[hp][c][:].rearrange(
                        "p (k f) -> p k f", k=2)[:, :, E:E + 1].squeeze(2)
                    nc.gpsimd.memset(ones_ap, 1.0)
            # r2[hp][c][:, k] = exp(-0.8 * a_n) of head hp*2+k (fp32: tensor
            # scalar max requires fp32 scalar operands)
            r2 = [[rv_pool.tile([P, 2], F32, tag=f"r{hp}_{c}", name=f"r{hp}_{c}")
                   for c in range(NCH)] for hp in range(HP)]
            v2 = [[rv_pool.tile([P, 2], F32, tag=f"v{hp}_{c}", name=f"v{hp}_{c}")
                   for c in range(NCH)] for hp in range(HP)]

            with (
                tc.tile_pool(name="proj_ps", bufs=2, space="PSUM") as proj_ps,
                tc.tile_pool(name="asrow_ps", bufs=2, space="PSUM") as asrow_ps,
                tc.tile_pool(name="att_ps", bufs=4, space="PSUM") as att_ps,
            ):
                # a_s rows -> w16row = Exp(0.8 * pa) fp16, then replicate
                # each head's row across partitions fully on-chip: PE ones
                # outer product into PSUM + ACT copy to fp16 SBUF.  (A DMA
                # round-trip through DRAM added ~10us of queue+sem latency.)
                # high_priority: this chain gates the very first DVE op, but
                # the static scheduler cannot see that.
                with tc.high_priority():
                    for s0, sw in segs:
                        pa = asrow_ps.tile([H, sw], F32, tag="pa",
                                           name=f"pa{s0}")
                        nc.tensor.matmul(
                            pa[:], kas_sb, xt_cols(s0, sw),
                            start=True, stop=True,
                        )
                        nc.scalar.activation(w16row[:, s0:s0 + sw], pa[:],
                                             mybir.ActivationFunctionType.Exp,
                                             scale=0.8)
                    # per-head rows at base partition 0 (matmul rhs req.)
                    for h in range(1, H):
                        nc.sync.dma_start(out=w16row0[h - 1][:],
                                          in_=w16row[h:h + 1, :])

                def bcast_w16(h):
                    row = w16row if h == 0 else w16row0[h - 1]
                    for s0, sw in segs:
                        bc = att_ps.tile([P, sw], F32, tag="att",
                                         name=f"bc{h}_{s0}")
                        nc.tensor.matmul(
                            bc[:], ones_sb[:],
                            row[0:1, s0:s0 + sw],
                            start=True, stop=True,
                        )
                        nc.scalar.copy(w16[h][:, s0:s0 + sw], bc[:])

                # only heads 0/1 gate early DVE work; heads 2/3 broadcast
                # later so their ACT copies don't delay the proj r/v exps
                with tc.high_priority():
                    bcast_w16(0)
                    bcast_w16(1)

                # adjacency loads: first quad sliced fine (needed ~12us in),
                # later quads coarser (fewer triggers)
                for c4 in range(NQ):
                    aseg = max(ASEG // 2, 256) if c4 == 0 else ASEG
                    for kq in range(QC):
                        c = c4 * QC + kq
                        for s in range(0, N, aseg):
                            nc.sync.dma_start(
                                out=adj_sb[c4][:, kq * N + s:kq * N + s + aseg],
                                in_=adjT[c * P:(c + 1) * P, s:s + aseg])

                # projection: feat + a_n columns, two heads per matmul
                for hp in range(HP):
                    for c in range(NCH):
                        ps = proj_ps.tile([P, 2 * EA], F32, tag="proj")
                        nc.tensor.matmul(
                            ps[:],
                            xt_cols(c * P, P),
                            kaug_sb[:, hp * 2 * EA:(hp + 1) * 2 * EA],
                            start=True, stop=True,
                        )
                        # psum cols: [feat_h0 | an_h0 | feat_h1 | an_h1]
                        ps3 = ps[:].rearrange("p (k f) -> p k f", k=2)
                        an_col = ps3[:, :, E:E + 1].squeeze(2)  # [P, 2]
                        f3 = feat2v[hp][c][:].rearrange("p (k f) -> p k f", k=2)
                        nc.scalar.activation(r2[hp][c][:], an_col,
                                             mybir.ActivationFunctionType.Exp,
                                             scale=-0.8)
                        nc.scalar.activation(v2[hp][c][:], an_col,
                                             mybir.ActivationFunctionType.Exp,
                                             bias=shift_sb[:])
                        # both heads' feat in one strided PSUM->SBUF copy
                        nc.scalar.copy(f3[:, :, 0:E], ps3[:, :, 0:E])
                    if hp == 0 and H > 2:
                        bcast_w16(2)
                        bcast_w16(3)

                # per-head attention
                with (
                    tc.tile_pool(name="q", bufs=6) as q_pool,
                    tc.tile_pool(name="p", bufs=2) as p_pool,
                    tc.tile_pool(name="ep", bufs=4) as ep_pool,
                ):
                    for h in range(H):
                        hp, k = h // 2, h % 2
                        # incremental accumulators for i-blocks 0..3
                        accs = []
                        if NCH == 8:
                            accs = [att_ps.tile([P, EA], F32, tag="att",
                                                name=f"atta{h}_{ib}")
                                    for ib in range(4)]
                            if h == H - 1:
                                # phase-1 pools are idle by the last head:
                                # run every i-block incrementally so almost
                                # no matmul trails the final mask op
                                accs += [proj_ps.tile([P, EA], F32, tag="proj",
                                                      name=f"atta{h}_{ib}")
                                         for ib in (4, 5)]
                                accs += [asrow_ps.tile([P, EA], F32, tag="pa",
                                                       name=f"atta{h}_{ib}")
                                        for ib in (6, 7)]
                        # per-chunk mask tts everywhere: measured ~460ns per
                        # [P,N] fp16 tt back-to-back, cheaper per element
                        # than one FD=4N quad op (~600ns/chunk), and finer
                        # tiles pipeline into the attention matmuls sooner
                        fine = (QC > 1)
                        alt_ep = (h == H - 1)
                        p_tiles = []   # (tile, col offset of chunk)
                        for c in range(NCH):
                            c4, kq = c // QC, c % QC
                            q_c = q_pool.tile([P, N], F16, tag="q",
                                              name=f"q{h}_{c}")
                            nc.vector.tensor_scalar(
                                q_c[:], w16[h][:],
                                r2[hp][c][:, k:k + 1],
                                v2[hp][c][:, k:k + 1],
                                mybir.AluOpType.max, mybir.AluOpType.mult)
                            p_c = p_pool.tile([P, N], F16, tag=f"pl{c}",
                                              name=f"p{h}_{c}")
                            nc.vector.tensor_tensor(
                                p_c[:], q_c[:],
                                adj_sb[c4][:, kq * N:(kq + 1) * N],
                                mybir.AluOpType.mult)
                            p_tiles.append((p_c, 0))
                            for ib, acc in enumerate(accs):
                                nc.tensor.matmul(
                                    acc[:],
                                    p_c[:, ib * P:(ib + 1) * P],
                                    feat2v[hp][c][:, k * EA:k * EA + EA],
                                    start=(c == 0), stop=(c == NCH - 1),
                                )

                        for ib in range(NCH):
                            if ib < len(accs):
                                acc = accs[ib]
                            else:
                                # trailing i-blocks borrow phase-1 PSUM banks
                                if ib < 6:
                                    acc = proj_ps.tile([P, EA], F32, tag="proj",
                                                       name=f"att{h}_{ib}")
                                else:
                                    acc = asrow_ps.tile([P, EA], F32, tag="pa",
                                                        name=f"att{h}_{ib}")
                                for c in range(NCH):
                                    pt, off = p_tiles[c]
                                    nc.tensor.matmul(
                                        acc[:],
                                        pt[:, off + ib * P:off + (ib + 1) * P],
                                        feat2v[hp][c][:, k * EA:k * EA + EA],
                                        start=(c == 0), stop=(c == NCH - 1),
                                    )
                            rec = ep_pool.tile([P, 1], F32, tag="rec",
                                               name=f"rec{h}_{ib}")
                            nc.vector.reciprocal(rec[:], acc[:, E:E + 1])
                            if ib == 0:
                                obh = [ep_pool.tile([P, NCH // 2 * E], F16,
                                                    tag=f"obh{half}", bufs=2,
                                                    name=f"obh{h}_{half}")
                                       for half in range(2)]
                            hb2 = NCH // 2
                            ob_ap = obh[ib // hb2][:, (ib % hb2) * E:
                                                   (ib % hb2 + 1) * E]
                            # relu(num * rec): rec > 0 so Relu(scale*x) works
                            if alt_ep and ib % 2 == 1:
                                # last head: alternate epilogues onto DVE so
                                # the final 8 don't serialize on ACT
                                nc.vector.tensor_scalar(
                                    ob_ap, acc[:, 0:E], rec[:], 0.0,
                                    mybir.AluOpType.mult, mybir.AluOpType.max)
                            else:
                                nc.scalar.activation(
                                    ob_ap, acc[:, 0:E],
                                    mybir.ActivationFunctionType.Relu,
                                    scale=rec[:])
                        # out DMAs: i-block halves normally; the last head
                        # writes per i-block pair so the final transfer is
                        # small (the 128KB half-DMA was a ~6us pure tail)
                        HB = NCH // 2
                        PW = max(HB // 2, 1) if alt_ep else HB
                        for half in range(2):
                            for q in range(HB // PW):
                                ib0 = half * HB + q * PW
                                nc.sync.dma_start(
                                    out=out[ib0 * P:(ib0 + PW) * P,
                                            h * E:(h + 1) * E].rearrange(
                                        "(ib r) c -> r ib c", r=P),
                                    in_=obh[half][:, q * PW * E:
                                                   (q + 1) * PW * E].rearrange(
                                        "p (ib c) -> p ib c", c=E))
    nc.compile()
    return nc


_PROGRAM_CACHE = {}


def _get_program(N, H):
    key = (N, H)
    if key not in _PROGRAM_CACHE:
        _PROGRAM_CACHE[key] = build_core_program(N, H)
    return _PROGRAM_CACHE[key]


def host_prep(x, adj, kernel, attn_self, attn_neigh):
    """Build per-core input maps (layout transforms + weight packing only)."""
    B, N, D = x.shape
    H, _, E = kernel.shape
    kaug = np.empty((D, H * (E + 1)), np.float32)
    kas = np.empty((D, H), np.float32)
    for h in range(H):
        kaug[:, h * (E + 1):h * (E + 1) + E] = kernel[h]
        kaug[:, h * (E + 1) + E] = kernel[h] @ attn_neigh[h]
        kas[:, h] = kernel[h] @ attn_self[h]
    in_maps = []
    for b in range(B):
        wx = np.concatenate(
            [kas, kaug, np.ascontiguousarray(x[b].T)], axis=1)
        in_maps.append({
            "wx": np.ascontiguousarray(wx).astype(np.float16),
            "adjT": np.ascontiguousarray(adj[b].T).astype(np.float16),
        })
    return in_maps


def kernel(x, adj, kernel, attn_self, attn_neigh, bias, _profile=None):
    x = np.asarray(x, np.float32)
    adj = np.asarray(adj, np.float32)
    kernel = np.asarray(kernel, np.float32)
    attn_self = np.asarray(attn_self, np.float32)
    attn_neigh = np.asarray(attn_neigh, np.float32)
    bias = np.asarray(bias, np.float32)

    B, N, D = x.shape
    H, _, E = kernel.shape
    nc = _get_program(N, H)
    in_maps = host_prep(x, adj, kernel, attn_self, attn_neigh)
    kwargs = dict(_profile) if _profile else {}
    last_err = None
    for _attempt in range(3):
        try:
            res = run_bass_kernel_spmd(nc, in_maps, list(range(B)), **kwargs)
            outs = np.stack(
                [np.asarray(res.results[b]["out"]).astype(np.float32)
                 for b in range(B)])
            break
        except Exception as exc:  # transient PJRT/axon fetch errors
            last_err = exc
    else:
        raise last_err
    assert not np.any(bias != 0.0), "nonzero-bias path not implemented"
    if _profile:
        return outs, res
    return outs


if __name__ == "__main__":
    # Mini smoke test: N=256, H=2, B=2 against a numpy reference.
    np.random.seed(0)
    N, H, D, E, B = 256, 2, 128, 128, 2
    LRELU_ALPHA = 0.2
    x = np.random.randn(B, N, D).astype(np.float32)
    adj = (np.random.rand(B, N, N) < 0.5).astype(np.float32)
    K = (np.random.randn(H, D, E) / np.sqrt(D)).astype(np.float32)
    a_s = (np.random.randn(H, E) / np.sqrt(E)).astype(np.float32)
    a_n = (np.random.randn(H, E) / np.sqrt(E)).astype(np.float32)
    bias = np.zeros((H, E), np.float32)

    def ref(x, adj, K, a_s, a_n, bias):
        feat = np.einsum('bnd,hde->bhne', x, K)
        s1 = np.einsum('bhne,he->bhn', feat, a_s)
        s2 = np.einsum('bhne,he->bhn', feat, a_n)
        sc = s1[..., :, None] + s2[..., None, :]
        sc = np.where(sc > 0, sc, LRELU_ALPHA * sc)
        sc = sc + (-1e10) * (1.0 - adj[:, None])
        sc = sc - sc.max(axis=-1, keepdims=True)
        att = np.exp(sc)
        att = att / att.sum(axis=-1, keepdims=True)
        o = np.einsum('bhnm,bhme->bhne', att, feat) + bias[None, :, None, :]
        o = o.transpose(0, 2, 1, 3).reshape(B, N, H * E)
        return np.maximum(o, 0.0)

    expected = ref(x, adj, K, a_s, a_n, bias)
    nc = _get_program(N, H)
    in_maps = host_prep(x, adj, K, a_s, a_n)
    res = run_bass_kernel_spmd(nc, in_maps, list(range(B)))
    actual = np.stack([np.asarray(res.results[b]["out"]).astype(np.float32)
                       for b in range(B)])
    err = np.abs(actual - expected).max() / np.abs(expected).max()
    rel = np.linalg.norm(actual - expected) / np.linalg.norm(expected)
    print(f"SMOKE absmax-rel: {err:.3e}  l2-rel: {rel:.3e}")

